# revision 54
# baseline (speedup 1.0000x reference)
"""Trainium2 Bass kernel for a pre-LN transformer block (B=2, T=2048, D=1024,
NH=16, HD=64, DFF=4096) on 8 NeuronCores.

Sharding: each core owns a contiguous 512-token slab of one batch (4 cores
per batch). Zero inter-core communication: every core recomputes K/V for its
whole batch (the only cross-token coupling), then computes attention + MLP
for its own slab only. The host rotates each core's batch tokens so the
owned slab sits at rows [0:512) -> one uniform SPMD program; causality is
carried by per-core mask data.

Precision: attention-branch matmuls (qkv, scores, att@v) in bf16; residual
stream matmuls (proj, fc1, fc2) in float32r; fp32 PSUM accumulation
everywhere. LayerNorm affine params are folded into adjacent matmul weights
on the host; ln1 statistics are computed via PE ones-matmuls in channel-major
space; softmax skips max-subtraction (logits bounded ~|2.6|); the softmax
denominator comes from a ones-column appended to V. All weights are
pre-tiled on the host so every DMA is contiguous.
"""

import sys

for _p in ("/opt/trn_rl_repo", "/root/.axon_site/_ro/trn_rl_repo"):
    if _p not in sys.path:
        sys.path.insert(0, _p)

import numpy as np
import ml_dtypes

import concourse.bass as bass
import concourse.tile as tile
from concourse import bacc, mybir
from concourse.bass_utils import run_bass_kernel_spmd

B = 2
T = 2048
D = 1024
NH = 16
HD = 64
DFF = 4 * D
EPS = 1e-5
P = 128
KO = D // P            # 8 contraction tiles for D
N_CORES = 8
CPB = N_CORES // B     # cores per batch
TC = T // CPB          # 512 own tokens per core
NT = T // P            # 16 token tiles per batch
NQT = TC // P          # 4 own-token tiles
FC = 512               # free-dim chunk for matmuls
NKT = T // P           # 16 key tiles
NFT = DFF // P         # 32 dff tiles

f32 = mybir.dt.float32
f32r = mybir.dt.float32r
bf16 = mybir.dt.bfloat16
f8 = mybir.dt.float8e4
AF = mybir.ActivationFunctionType
ALU = mybir.AluOpType
DR = mybir.MatmulPerfMode.DoubleRow
NFP = NFT // 2          # 16 dff 128-pair tiles for fc2

_CACHE = {}
_ONLY_A = False


def build_nc():
    nc = bacc.Bacc("TRN2", target_bir_lowering=False)

    io = {}
    d = nc.declare_dram_parameter
    io["xbT"] = d("xbT", [D, T], bf16, isOutput=False)       # rotated x, transposed
    io["x_own"] = d("x_own", [TC, D], f32, isOutput=False)
    io["w_k"] = d("w_k", [P, 16, KO // 2, 2, 64], f8, isOutput=False)
    io["w_v"] = d("w_v", [P, KO // 2, 2, D], f8, isOutput=False)
    io["w_q"] = d("w_q", [P, 16, KO // 2, 2, 64], f8, isOutput=False)
    io["b_k"] = d("b_k", [64, 16], f32, isOutput=False)
    io["b_q"] = d("b_q", [64, 16], f32, isOutput=False)
    io["w_proj"] = d("w_proj", [P, KO, D], bf16, isOutput=False)
    io["fc1_w"] = d("fc1_w", [P, NFT, KO, P], bf16, isOutput=False)
    io["fc1_b"] = d("fc1_b", [P, NFT], f32, isOutput=False)
    io["fc2_w"] = d("fc2_w", [P, 2, NFP, 2, FC], f8, isOutput=False)
    io["fc2_b_rep"] = d("fc2_b_rep", [P, D], f32, isOutput=False)
    io["mask"] = d("mask", [P, NKT, TC], bf16, isOutput=False)
    io["ident"] = d("ident", [P, P], f32r, isOutput=False)
    io["identb"] = d("identb", [P, P], bf16, isOutput=False)
    io["ones"] = d("ones", [P, 1], bf16, isOutput=False)
    io["out"] = d("out", [TC, D], f32, isOutput=True)

    io["bc_scratch"] = nc.dram_tensor("bc_scratch", [2, T], bf16)
    with tile.TileContext(nc) as tc:
        _emit(nc, tc, io)
    nc.compile()
    return nc


def _emit(nc, tc, io):
    from contextlib import ExitStack

    with ExitStack() as ctx:
        singles = ctx.enter_context(tc.tile_pool(name="singles", bufs=1))
        psA_cm = tc.tile_pool(name="psA", bufs=2, space="PSUM")
        psA = psA_cm.__enter__()
        psB_cm = tc.tile_pool(name="psB", bufs=2, space="PSUM")
        psB = psB_cm.__enter__()

        ident = singles.tile([P, P], f32r)
        nc.sync.dma_start(out=ident, in_=io["ident"].ap())
        identb = singles.tile([P, P], bf16)
        nc.sync.dma_start(out=identb, in_=io["identb"].ap())
        ones = singles.tile([P, 1], bf16)
        nc.sync.dma_start(out=ones, in_=io["ones"].ap())

        pOut = ctx.enter_context(tc.tile_pool(name="pOut", bufs=1))
        attn_out = pOut.tile([P, NQT, D], bf16)
        xo_all = pOut.tile([P, NQT, D], f32)

        pABC_cm = tc.tile_pool(name="pABC", bufs=1)
        pABC = pABC_cm.__enter__()
        kT = pABC.tile([P, KO, T], bf16)
        v_aug = pABC.tile([P, NT, NH, HD + 1], bf16)
        qT = pABC.tile([P, KO, TC], bf16)
        nc.vector.memset(v_aug[:, :, :, HD:HD + 1], 1.0)

        # ---------- Phase A: ln1 in channel-major space ----------
        pXN_cm = tc.tile_pool(name="pXN", bufs=1)
        pXN = pXN_cm.__enter__()
        xbT = pXN.tile([P, KO, T], bf16)
        xnT = xbT
        pA2_cm = tc.tile_pool(name="pA2", bufs=1)
        pA2 = pA2_cm.__enter__()
        for ch in range(T // FC):
            for ko in range(KO):
                nc.sync.dma_start(
                    out=xbT[:, ko, ch * FC:(ch + 1) * FC],
                    in_=io["xbT"].ap()[ko * P:(ko + 1) * P, ch * FC:(ch + 1) * FC])
        r_bc = pA2.tile([P, T], bf16)
        nmr_bc = pA2.tile([P, T], bf16)
        with tc.tile_pool(name="ln1", bufs=1) as ln1p, \
             tc.tile_pool(name="sqp", bufs=3) as sqp:
            for ch in range(T // FC):
                st_ps = psA.tile([33, FC], f32, tag="t1")
                s_ps = st_ps[0:1, :]
                q_ps = st_ps[32:33, :]
                for ko in range(KO):
                    nc.tensor.matmul(s_ps, ones, xbT[:, ko, ch * FC:(ch + 1) * FC],
                                     start=(ko == 0), stop=(ko == KO - 1))
                for ko in range(KO):
                    sq = sqp.tile([P, FC], bf16, tag="sq")
                    nc.scalar.activation(out=sq, func=AF.Square,
                                         in_=xbT[:, ko, ch * FC:(ch + 1) * FC])
                    nc.tensor.matmul(q_ps, ones, sq,
                                     start=(ko == 0), stop=(ko == KO - 1))
                mu = ln1p.tile([1, FC], f32, tag="mu")
                nc.vector.tensor_scalar(out=mu, in0=s_ps, scalar1=1.0 / D,
                                        scalar2=None, op0=ALU.mult)
                var = ln1p.tile([1, FC], f32, tag="var")
                nc.vector.tensor_scalar(out=var, in0=q_ps, scalar1=1.0 / D,
                                        scalar2=None, op0=ALU.mult)
                tmp = ln1p.tile([1, FC], f32, tag="tmp")
                nc.vector.tensor_mul(out=tmp, in0=mu, in1=mu)
                nc.vector.tensor_tensor(out=var, in0=var, in1=tmp,
                                        op=ALU.subtract)
                nc.vector.tensor_scalar(out=var, in0=var, scalar1=EPS,
                                        scalar2=None, op0=ALU.add)
                nc.scalar.activation(out=var, in_=var, func=AF.Sqrt)
                nc.vector.reciprocal(out=tmp, in_=var)       # tmp = rstd
                nc.vector.tensor_mul(out=mu, in0=mu, in1=tmp)
                nc.vector.tensor_scalar(out=mu, in0=mu, scalar1=-1.0,
                                        scalar2=None, op0=ALU.mult)  # mu = -mu*rstd
                rb16 = ln1p.tile([1, FC], bf16, tag="rb16")
                nc.vector.tensor_copy(out=rb16, in_=tmp)
                nb16 = ln1p.tile([1, FC], bf16, tag="nb16")
                nc.vector.tensor_copy(out=nb16, in_=mu)
                # broadcast across the 128 partitions via DRAM bounce
                bcs = io["bc_scratch"]
                nc.sync.dma_start(out=bcs.ap()[0:1, ch * FC:(ch + 1) * FC],
                                  in_=rb16)
                nc.sync.dma_start(out=bcs.ap()[1:2, ch * FC:(ch + 1) * FC],
                                  in_=nb16)
                nc.sync.dma_start(
                    out=r_bc[:, ch * FC:(ch + 1) * FC],
                    in_=bass.AP(tensor=bcs, offset=ch * FC,
                                ap=[[0, P], [1, FC]]))
                nc.sync.dma_start(
                    out=nmr_bc[:, ch * FC:(ch + 1) * FC],
                    in_=bass.AP(tensor=bcs, offset=T + ch * FC,
                                ap=[[0, P], [1, FC]]))
            # xnT = xbT * r + (-mu*r), chunked for pipelining with phase B
            for ch in range(T // FC):
                s = slice(ch * FC, (ch + 1) * FC)
                for ko in range(KO):
                    eng = nc.gpsimd if ko < 2 else nc.vector
                    eng.tensor_mul(out=xnT[:, ko, s], in0=xbT[:, ko, s],
                                   in1=r_bc[:, s])
                    eng.tensor_add(out=xnT[:, ko, s], in0=xnT[:, ko, s],
                                   in1=nmr_bc[:, s])
        pA2_cm.__exit__(None, None, None)

        if _ONLY_A:
            with tc.tile_pool(name="dumA", bufs=2) as dp:
                for qt in range(NQT):
                    t0 = dp.tile([P, KO, P], f32, tag="t0")
                    nc.vector.tensor_copy(out=t0, in_=xnT[:, :, qt * P:(qt + 1) * P])
                    nc.sync.dma_start(
                        out=io["out"].ap()[qt * P:(qt + 1) * P, :],
                        in_=t0.rearrange("p ko t -> p (ko t)"))
            pABC_cm.__exit__(None, None, None)
            return

        # ---------- Phase B: Q^T, V rows, then K^T (fp8 DoubleRow) ----------
        pB2_cm = tc.tile_pool(name="pB2", bufs=1)
        pB2 = pB2_cm.__enter__()
        wv_sb = pB2.tile([P, KO // 2, 2, D], f8)
        nc.sync.dma_start(out=wv_sb, in_=io["w_v"].ap())
        bk_sb = pB2.tile([64, 16], f32)
        nc.sync.dma_start(out=bk_sb, in_=io["b_k"].ap())
        bq_sb = pB2.tile([64, 16], f32)
        nc.sync.dma_start(out=bq_sb, in_=io["b_q"].ap())
        wk_sb = pB2.tile([P, 16, KO // 2, 2, 64], f8)
        nc.sync.dma_start(out=wk_sb, in_=io["w_k"].ap())
        wq_sb = pB2.tile([P, 16, KO // 2, 2, 64], f8)
        nc.sync.dma_start(out=wq_sb, in_=io["w_q"].ap())
        xn8 = pB2.tile([P, KO, T], f8)

        with tc.tile_pool(name="wkv", bufs=3) as wp, \
             tc.tile_pool(name="kst", bufs=3) as kstp:
            # quantize xn to fp8 for the K DoubleRow matmuls (Act is idle here)
            for ko in range(KO):
                for ch in range(T // FC):
                    nc.scalar.copy(out=xn8[:, ko, ch * FC:(ch + 1) * FC],
                                   in_=xnT[:, ko, ch * FC:(ch + 1) * FC])
            for t64 in range(NT * 2):  # V rows, fp8 DoubleRow (64-tok tiles)
                tt, hi = t64 // 2, t64 % 2
                for vc in range(2):
                    acc = psB.tile([64, FC], f32, tag="t2")
                    for kp in range(KO // 2):
                        nc.tensor.matmul(
                            acc,
                            xn8[:, 2 * kp:2 * kp + 2,
                                t64 * 64:(t64 + 1) * 64],
                            wv_sb[:, kp, :, vc * FC:(vc + 1) * FC],
                            start=(kp == 0), stop=(kp == KO // 2 - 1),
                            perf_mode=DR)
                    hs = vc * 8
                    if hi == 0:
                        nc.scalar.activation(
                            out=v_aug[0:64, tt, hs:hs + 8, 0:HD],
                            in_=acc.rearrange("p (h d) -> p h d", h=8),
                            func=AF.Identity, scale=1.0 / 256.0)
                    else:
                        vst = kstp.tile([64, FC], bf16, tag="vst")
                        nc.vector.tensor_scalar(
                            out=vst, in0=acc, scalar1=1.0 / 256.0,
                            scalar2=None, op0=ALU.mult)
                        nc.sync.dma_start(
                            out=v_aug[64:128, tt, hs:hs + 8, 0:HD],
                            in_=vst.rearrange("p (h d) -> p h d", h=8))
            # K^T via fp8 DoubleRow: 16 ct64 tiles, odd tiles DMA-shifted
            # into partitions 64-127 of kT
            for ct in range(16):
                ct128, hi = ct // 2, ct % 2
                # interleaved Q^T (fp8 DR) keeps the PE stream dense
                qacc = psB.tile([64, TC], f32, tag="t2")
                for kp in range(KO // 2):
                    nc.tensor.matmul(qacc, wq_sb[:, ct, kp, :, :],
                                     xn8[:, 2 * kp:2 * kp + 2, 0:TC],
                                     start=(kp == 0), stop=(kp == KO // 2 - 1),
                                     perf_mode=DR)
                if hi == 0:
                    nc.scalar.activation(out=qT[0:64, ct128, :], in_=qacc,
                                         func=AF.Identity,
                                         bias=bq_sb[:, ct:ct + 1],
                                         scale=1.0 / 256.0)
                else:
                    qst = kstp.tile([64, TC], bf16, tag="qst")
                    nc.vector.tensor_scalar(
                        out=qst, in0=qacc, scalar1=1.0 / 256.0,
                        scalar2=bq_sb[:, ct:ct + 1],
                        op0=ALU.mult, op1=ALU.add)
                    nc.sync.dma_start(out=qT[64:128, ct128, :], in_=qst)
                for np_ in range(T // FC):
                    acc = psA.tile([64, FC], f32, tag="t1")
                    for kp in range(KO // 2):
                        nc.tensor.matmul(
                            acc, wk_sb[:, ct, kp, :, :],
                            xn8[:, 2 * kp:2 * kp + 2,
                                np_ * FC:(np_ + 1) * FC],
                            start=(kp == 0), stop=(kp == KO // 2 - 1),
                            perf_mode=DR)
                    if hi == 0:
                        nc.scalar.activation(
                            out=kT[0:64, ct128, np_ * FC:(np_ + 1) * FC],
                            in_=acc, func=AF.Identity,
                            bias=bk_sb[:, ct:ct + 1], scale=1.0 / 256.0)
                    else:
                        kst = kstp.tile([64, FC], bf16, tag="kst")
                        nc.vector.tensor_scalar(
                            out=kst, in0=acc, scalar1=1.0 / 256.0,
                            scalar2=bk_sb[:, ct:ct + 1],
                            op0=ALU.mult, op1=ALU.add)
                        nc.sync.dma_start(
                            out=kT[64:128, ct128, np_ * FC:(np_ + 1) * FC],
                            in_=kst)
        pB2_cm.__exit__(None, None, None)
        pXN_cm.__exit__(None, None, None)
        for qt in range(NQT):
            nc.sync.dma_start(out=xo_all[:, qt, :],
                              in_=io["x_own"].ap()[qt * P:(qt + 1) * P, :])

        # ---------- Phase C: attention ----------
        pC2_cm = tc.tile_pool(name="pC2", bufs=1)
        pC2 = pC2_cm.__enter__()
        mask_sb = pC2.tile([P, NKT, TC], bf16)
        nc.sync.dma_start(out=mask_sb, in_=io["mask"].ap())
        with tc.tile_pool(name="attp", bufs=2) as attp:
            for h in range(NH):
                hp = (h * HD) % P
                hko = (h * HD) // P
                attT = attp.tile([P, NKT, TC], bf16)
                for kp in range(NKT // 2):
                    sc = psA.tile([P, 2, TC], f32, tag="t1")
                    for half in range(2):
                        kt = kp * 2 + half
                        nc.tensor.matmul(sc[:, half, :],
                                         kT[hp:hp + HD, hko, kt * P:(kt + 1) * P],
                                         qT[hp:hp + HD, hko, :],
                                         start=True, stop=True)
                    nc.scalar.activation(out=attT[:, kp * 2:(kp + 1) * 2, :],
                                         in_=sc, func=AF.Exp)
                nc.vector.tensor_mul(out=attT[:, :, :], in0=attT[:, :, :],
                                     in1=mask_sb[:, :, :])
                for qt in range(NQT):
                    av = psB.tile([P, HD + 1], f32, tag="t2")
                    for kt in range(NKT):
                        nc.tensor.matmul(av,
                                         attT[:, kt, qt * P:(qt + 1) * P],
                                         v_aug[:, kt, h, :],
                                         start=(kt == 0), stop=(kt == NKT - 1))
                    recip = attp.tile([P, 1], f32, tag="recip")
                    nc.vector.reciprocal(out=recip, in_=av[:, HD:HD + 1])
                    nc.vector.tensor_scalar(
                        out=attn_out[:, qt, h * HD:(h + 1) * HD],
                        in0=av[:, 0:HD], scalar1=recip, scalar2=None, op0=ALU.mult)

        pC2_cm.__exit__(None, None, None)
        pABC_cm.__exit__(None, None, None)

        # ---------- Phase D: proj + residual -> x2; ln2 -> xn2T ----------
        pDE_cm = tc.tile_pool(name="pDE", bufs=1)
        pDE = pDE_cm.__enter__()
        pE_cm = tc.tile_pool(name="pE", bufs=1)
        pE = pE_cm.__enter__()
        pW_cm = tc.tile_pool(name="pW", bufs=1)
        pW = pW_cm.__enter__()
        x2 = pDE.tile([P, NQT, D], f32)
        xn2T = pDE.tile([P, KO, TC], bf16)
        wproj_sb = pW.tile([P, KO, D], bf16)
        nc.sync.dma_start(out=wproj_sb, in_=io["w_proj"].ap())
        w1 = pE.tile([P, NFT, KO, P], bf16)
        nc.sync.dma_start(out=w1, in_=io["fc1_w"].ap())
        w2a = pE.tile([P, NFP, 2, FC], f8)
        nc.sync.dma_start(out=w2a, in_=io["fc2_w"].ap()[:, 0])
        w2b = pE.tile([P, NFP, 2, FC], f8)
        nc.sync.dma_start(out=w2b, in_=io["fc2_w"].ap()[:, 1])
        b1_sb = pE.tile([P, NFT], f32)
        nc.sync.dma_start(out=b1_sb, in_=io["fc1_b"].ap())
        b2rep = pE.tile([P, D], f32)
        nc.sync.dma_start(out=b2rep, in_=io["fc2_b_rep"].ap())
        eps_t = pDE.tile([P, 1], f32)
        nc.vector.memset(eps_t, EPS)

        with tc.tile_pool(name="projp", bufs=3) as pp:
            attn_outT = pW.tile([P, KO, TC], bf16)
            for qt in range(NQT):
                for ko in range(KO):
                    tp = psA.tile([P, P], bf16, tag="t1")
                    nc.tensor.transpose(
                        tp, attn_out[:, qt, ko * P:(ko + 1) * P], identb)
                    nc.vector.tensor_copy(
                        out=attn_outT[:, ko, qt * P:(qt + 1) * P], in_=tp)
            for qt in range(NQT):
                xo = xo_all[:, qt, :]
                for oc in range(D // FC):
                    acc = psB.tile([P, FC], f32, tag="t2")
                    for ko in range(KO):
                        nc.tensor.matmul(acc, attn_outT[:, ko, qt * P:(qt + 1) * P],
                                         wproj_sb[:, ko, oc * FC:(oc + 1) * FC],
                                         start=(ko == 0), stop=(ko == KO - 1))
                    nc.vector.tensor_add(out=x2[:, qt, oc * FC:(oc + 1) * FC],
                                         in0=acc, in1=xo[:, oc * FC:(oc + 1) * FC])
                stats = pp.tile([P, 2, 6], f32, tag="st2")
                nc.vector.bn_stats(out=stats[:, 0, :], in_=x2[:, qt, 0:512])
                nc.vector.bn_stats(out=stats[:, 1, :], in_=x2[:, qt, 512:1024])
                mv = pp.tile([P, 2], f32, tag="mv2")
                nc.vector.bn_aggr(out=mv, in_=stats)
                std = pp.tile([P, 1], f32, tag="sd2")
                nc.scalar.activation(out=std, in_=mv[:, 1:2], func=AF.Sqrt,
                                     bias=eps_t, scale=1.0)
                rstd = pp.tile([P, 1], f32, tag="rs2")
                nc.vector.reciprocal(out=rstd, in_=std)
                xn2 = pp.tile([P, D], bf16, tag="xn2")
                nc.vector.tensor_scalar(out=xn2, in0=x2[:, qt, :], scalar1=mv[:, 0:1],
                                        scalar2=rstd, op0=ALU.subtract, op1=ALU.mult)
                for ko in range(KO):
                    tp = psA.tile([P, P], bf16, tag="t1")
                    nc.tensor.transpose(tp, xn2[:, ko * P:(ko + 1) * P], identb)
                    nc.vector.tensor_copy(
                        out=xn2T[:, ko, qt * P:(qt + 1) * P], in_=tp)

        pW_cm.__exit__(None, None, None)

        psB_cm.__exit__(None, None, None)
        psA_cm.__exit__(None, None, None)
        # ---------- Phase E: fc1 -> gelu -> hT; fc2 (fp8 DoubleRow) ----------
        hT = pE.tile([P, NFT, TC], f8)
        x2b = pE.tile([P, NQT, D], bf16)
        for qt in range(NQT):
            for oc in range(2):
                nc.vector.scalar_tensor_tensor(
                    out=x2b[:, qt, oc * FC:(oc + 1) * FC],
                    in0=x2[:, qt, oc * FC:(oc + 1) * FC],
                    scalar=256.0, op0=ALU.mult,
                    in1=b2rep[:, oc * FC:(oc + 1) * FC], op1=ALU.add)
        with tc.tile_pool(name="f1ps", bufs=4, space="PSUM") as f1ps:
            for ft in range(NFT):
                acc = f1ps.tile([P, TC], f32, tag="facc1")
                for ko in range(KO):
                    nc.tensor.matmul(acc, w1[:, ft, ko, :], xn2T[:, ko, :],
                                     start=(ko == 0), stop=(ko == KO - 1))
                nc.scalar.activation(out=hT[:, ft, :], in_=acc, func=AF.Gelu,
                                     bias=b1_sb[:, ft:ft + 1], scale=1.0)

        with tc.tile_pool(name="fc2p", bufs=4, space="PSUM") as f2ps, \
             tc.tile_pool(name="fc2o", bufs=4) as f2p:
            for oc in range(2):
                w2 = w2a if oc == 0 else w2b
                for q64 in range(TC // 64):
                    acc = f2ps.tile([64, FC], f32, tag="facc")
                    for i in range(NFP):
                        nc.tensor.matmul(
                            acc,
                            hT[:, 2 * i:2 * i + 2, q64 * 64:(q64 + 1) * 64],
                            w2[:, i, :, :],
                            start=(i == 0), stop=False,
                            perf_mode=DR)
                    qt, h2 = q64 // 2, q64 % 2
                    nc.tensor.matmul(
                        acc,
                        identb[h2 * 64:h2 * 64 + 64, h2 * 64:h2 * 64 + 64],
                        x2b[h2 * 64:h2 * 64 + 64, qt, oc * FC:(oc + 1) * FC],
                        start=False, stop=True)
                    osb = f2p.tile([64, FC], f32, tag="osb")
                    nc.vector.tensor_scalar(out=osb, in0=acc, scalar1=1.0 / 256.0,
                                            scalar2=None, op0=ALU.mult)
                    nc.sync.dma_start(
                        out=io["out"].ap()[q64 * 64:(q64 + 1) * 64,
                                           oc * FC:(oc + 1) * FC],
                        in_=osb)
        pE_cm.__exit__(None, None, None)
        pDE_cm.__exit__(None, None, None)


def _stage_inputs(x, w_qkv, w_proj, ln1_w, ln1_b, ln2_w, ln2_b,
                  fc1_w, fc1_b, fc2_w, fc2_b):
    """Build the 8 per-core input maps (host-side sharding/folding/tiling)."""
    f = np.float32
    bf = ml_dtypes.bfloat16
    x = np.asarray(x, f)
    w_qkv = np.asarray(w_qkv, f)
    ln1_w, ln1_b = np.asarray(ln1_w, f), np.asarray(ln1_b, f)
    ln2_w, ln2_b = np.asarray(ln2_w, f), np.asarray(ln2_b, f)
    fc1_wf, fc1_bf = np.asarray(fc1_w, f), np.asarray(fc1_b, f)
    fc2_wf, fc2_bf = np.asarray(fc2_w, f), np.asarray(fc2_b, f)
    w_projf = np.asarray(w_proj, f)

    wq_f = ln1_w[:, None] * w_qkv
    bq_f = ln1_b @ w_qkv
    scale = 1.0 / np.sqrt(HD)

    def tile_cols(w, dtype):
        # [D, M] -> [M/P, P, KO, P]: out[ct, p, ko, m] = w[ko*P+p, ct*P+m]
        Din, M = w.shape
        return np.ascontiguousarray(
            w.reshape(KO, P, M // P, P).transpose(2, 1, 0, 3)).astype(dtype)

    f8q = ml_dtypes.float8_e4m3fn
    w_q_h = np.ascontiguousarray(
        (wq_f[:, 0:D] * scale * 256.0).reshape(KO // 2, 2, P, 16, 64)
        .transpose(2, 3, 0, 1, 4)).astype(f8q)
    b_q_h = np.ascontiguousarray(
        (bq_f[0:D] * scale).reshape(16, 64).T).astype(f)
    f8np_ = ml_dtypes.float8_e4m3fn
    # K weights as fp8 DoubleRow tiles [p, ct64, kp, j, m], scaled x256
    w_k_h = np.ascontiguousarray(
        (wq_f[:, D:2 * D] * 256.0).reshape(KO // 2, 2, P, 16, 64)
        .transpose(2, 3, 0, 1, 4)).astype(f8np_)
    b_k_h = np.ascontiguousarray(bq_f[D:2 * D].reshape(16, 64).T).astype(f)
    # V weights fp8 DR [p, kp, j, c], scaled x256
    w_v_h = np.ascontiguousarray(
        (wq_f[:, 2 * D:3 * D] * 256.0).reshape(KO // 2, 2, P, D)
        .transpose(2, 0, 1, 3)).astype(ml_dtypes.float8_e4m3fn)
    b_v = bq_f[2 * D:3 * D]
    bvproj = b_v @ w_projf

    w_proj_h = np.ascontiguousarray(
        w_projf.reshape(KO, P, D).transpose(1, 0, 2)).astype(bf)
    f8np = ml_dtypes.float8_e4m3fn
    fc1_w_h = np.ascontiguousarray(
        tile_cols(ln2_w[:, None] * fc1_wf, bf).transpose(1, 0, 2, 3))
    fc1_b_h = np.ascontiguousarray(
        (ln2_b @ fc1_wf + fc1_bf).reshape(NFT, P).T).astype(f)
    # fc2 DoubleRow tiles [i, oc, P, 2, FC]:
    #   fc2_h[i, oc, p, j, m] = fc2[(2*i+j)*P + p, oc*FC + m]
    # [p, oc, i, j, c] = fc2[(2i+j)*P + p, oc*FC + c]
    fc2_h = np.ascontiguousarray(
        fc2_wf.reshape(NFP, 2, P, 2, FC).transpose(2, 3, 0, 1, 4) * 256.0).astype(f8np)
    fc2_b_rep_h = np.ascontiguousarray(
        np.broadcast_to(fc2_bf * 256.0, (P, D))).astype(f)
    eye = np.eye(P, dtype=f)
    eyeb = np.eye(P, dtype=f).astype(bf)
    ones_h = np.ones((P, 1), f).astype(bf)

    in_maps = []
    for c in range(N_CORES):
        b = c // CPB
        r0 = (c % CPB) * TC
        xb_c = np.roll(x[b], -r0, axis=0)
        xbT_c = np.ascontiguousarray(xb_c.T).astype(bf)
        x_own_c = (x[b, r0:r0 + TC] + bvproj).astype(f)
        kidx = (np.arange(T) + r0) % T
        qidx = r0 + np.arange(TC)
        m = (kidx[:, None] <= qidx[None, :])
        mask_c = np.ascontiguousarray(
            m.reshape(NKT, P, TC).transpose(1, 0, 2)).astype(bf)
        in_maps.append({
            "xbT": xbT_c,
            "x_own": x_own_c,
            "w_k": w_k_h,
            "w_v": w_v_h,
            "w_q": w_q_h,
            "b_k": b_k_h,
            "b_q": b_q_h,
            "w_proj": w_proj_h,
            "fc1_w": fc1_w_h,
            "fc1_b": fc1_b_h,
            "fc2_w": fc2_h,
            "fc2_b_rep": fc2_b_rep_h,
            "mask": mask_c,
            "ident": eye,
            "identb": eyeb,
            "ones": ones_h,
        })
    return in_maps


def kernel(**inputs) -> np.ndarray:
    if "nc" not in _CACHE:
        _CACHE["nc"] = build_nc()
    nc = _CACHE["nc"]
    in_maps = _stage_inputs(**inputs)
    res = run_bass_kernel_spmd(nc, in_maps, list(range(N_CORES)))
    out = np.empty((B, T, D), np.float32)
    for c in range(N_CORES):
        b = c // CPB
        r0 = (c % CPB) * TC
        out[b, r0:r0 + TC] = res.results[c]["out"]
    return out



# revision 55
# speedup vs baseline: 1.0058x; 1.0058x over previous
"""Trainium2 Bass kernel for a pre-LN transformer block (B=2, T=2048, D=1024,
NH=16, HD=64, DFF=4096) on 8 NeuronCores.

Sharding: each core owns a contiguous 512-token slab of one batch (4 cores
per batch). Zero inter-core communication: every core recomputes K/V for its
whole batch (the only cross-token coupling), then computes attention + MLP
for its own slab only. The host rotates each core's batch tokens so the
owned slab sits at rows [0:512) -> one uniform SPMD program; causality is
carried by per-core mask data.

Precision: attention-branch matmuls (qkv, scores, att@v) in bf16; residual
stream matmuls (proj, fc1, fc2) in float32r; fp32 PSUM accumulation
everywhere. LayerNorm affine params are folded into adjacent matmul weights
on the host; ln1 statistics are computed via PE ones-matmuls in channel-major
space; softmax skips max-subtraction (logits bounded ~|2.6|); the softmax
denominator comes from a ones-column appended to V. All weights are
pre-tiled on the host so every DMA is contiguous.
"""

import sys

for _p in ("/opt/trn_rl_repo", "/root/.axon_site/_ro/trn_rl_repo"):
    if _p not in sys.path:
        sys.path.insert(0, _p)

import numpy as np
import ml_dtypes

import concourse.bass as bass
import concourse.tile as tile
from concourse import bacc, mybir
from concourse.bass_utils import run_bass_kernel_spmd

B = 2
T = 2048
D = 1024
NH = 16
HD = 64
DFF = 4 * D
EPS = 1e-5
P = 128
KO = D // P            # 8 contraction tiles for D
N_CORES = 8
CPB = N_CORES // B     # cores per batch
TC = T // CPB          # 512 own tokens per core
NT = T // P            # 16 token tiles per batch
NQT = TC // P          # 4 own-token tiles
FC = 512               # free-dim chunk for matmuls
NKT = T // P           # 16 key tiles
NFT = DFF // P         # 32 dff tiles

f32 = mybir.dt.float32
f32r = mybir.dt.float32r
bf16 = mybir.dt.bfloat16
f8 = mybir.dt.float8e4
AF = mybir.ActivationFunctionType
ALU = mybir.AluOpType
DR = mybir.MatmulPerfMode.DoubleRow
NFP = NFT // 2          # 16 dff 128-pair tiles for fc2

_CACHE = {}
_ONLY_A = False


def build_nc():
    nc = bacc.Bacc("TRN2", target_bir_lowering=False)

    io = {}
    d = nc.declare_dram_parameter
    io["xbT"] = d("xbT", [D, T], bf16, isOutput=False)       # rotated x, transposed
    io["x_own"] = d("x_own", [TC, D], f32, isOutput=False)
    io["w_k"] = d("w_k", [P, 16, KO // 2, 2, 64], f8, isOutput=False)
    io["w_v"] = d("w_v", [P, KO // 2, 2, D], f8, isOutput=False)
    io["w_q"] = d("w_q", [P, 16, KO // 2, 2, 64], f8, isOutput=False)
    io["b_k"] = d("b_k", [64, 16], f32, isOutput=False)
    io["b_q"] = d("b_q", [64, 16], f32, isOutput=False)
    io["w_proj"] = d("w_proj", [P, KO, D], bf16, isOutput=False)
    io["fc1_w"] = d("fc1_w", [P, NFT, KO, P], bf16, isOutput=False)
    io["fc1_b"] = d("fc1_b", [P, NFT], f32, isOutput=False)
    io["fc2_w"] = d("fc2_w", [P, 2, NFP, 2, FC], f8, isOutput=False)
    io["fc2_b_rep"] = d("fc2_b_rep", [P, D], f32, isOutput=False)
    io["mask"] = d("mask", [P, NKT, TC], bf16, isOutput=False)
    io["ident"] = d("ident", [P, P], f32r, isOutput=False)
    io["identb"] = d("identb", [P, P], bf16, isOutput=False)
    io["ones"] = d("ones", [P, 1], bf16, isOutput=False)
    io["out"] = d("out", [TC, D], f32, isOutput=True)

    io["bc_scratch"] = nc.dram_tensor("bc_scratch", [2, T], bf16)
    with tile.TileContext(nc) as tc:
        _emit(nc, tc, io)
    nc.compile()
    return nc


def _emit(nc, tc, io):
    from contextlib import ExitStack

    with ExitStack() as ctx:
        singles = ctx.enter_context(tc.tile_pool(name="singles", bufs=1))
        psA_cm = tc.tile_pool(name="psA", bufs=2, space="PSUM")
        psA = psA_cm.__enter__()
        psB_cm = tc.tile_pool(name="psB", bufs=2, space="PSUM")
        psB = psB_cm.__enter__()

        ident = singles.tile([P, P], f32r)
        nc.sync.dma_start(out=ident, in_=io["ident"].ap())
        identb = singles.tile([P, P], bf16)
        nc.sync.dma_start(out=identb, in_=io["identb"].ap())
        ones = singles.tile([P, 1], bf16)
        nc.sync.dma_start(out=ones, in_=io["ones"].ap())

        pOut = ctx.enter_context(tc.tile_pool(name="pOut", bufs=1))
        attn_out = pOut.tile([P, NQT, D], bf16)
        xo_all = pOut.tile([P, NQT, D], f32)

        pABC_cm = tc.tile_pool(name="pABC", bufs=1)
        pABC = pABC_cm.__enter__()
        kT = pABC.tile([P, KO, T], bf16)
        v_aug = pABC.tile([P, NT, NH, HD + 1], bf16)
        qT = pABC.tile([P, KO, TC], bf16)
        nc.vector.memset(v_aug[:, :, :, HD:HD + 1], 1.0)

        # ---------- Phase A: ln1 in channel-major space ----------
        pXN_cm = tc.tile_pool(name="pXN", bufs=1)
        pXN = pXN_cm.__enter__()
        xbT = pXN.tile([P, KO, T], bf16)
        xnT = xbT
        pA2_cm = tc.tile_pool(name="pA2", bufs=1)
        pA2 = pA2_cm.__enter__()
        for ch in range(T // FC):
            for ko in range(KO):
                nc.sync.dma_start(
                    out=xbT[:, ko, ch * FC:(ch + 1) * FC],
                    in_=io["xbT"].ap()[ko * P:(ko + 1) * P, ch * FC:(ch + 1) * FC])
        r_bc = pA2.tile([P, T], bf16)
        nmr_bc = pA2.tile([P, T], bf16)
        with tc.tile_pool(name="ln1", bufs=2) as ln1p, \
             tc.tile_pool(name="sqp", bufs=3) as sqp:
            for ch in range(T // FC):
                st_ps = psA.tile([33, FC], f32, tag="t1")
                s_ps = st_ps[0:1, :]
                q_ps = st_ps[32:33, :]
                for ko in range(KO):
                    nc.tensor.matmul(s_ps, ones, xbT[:, ko, ch * FC:(ch + 1) * FC],
                                     start=(ko == 0), stop=(ko == KO - 1))
                for ko in range(KO):
                    sq = sqp.tile([P, FC], bf16, tag="sq")
                    nc.scalar.activation(out=sq, func=AF.Square,
                                         in_=xbT[:, ko, ch * FC:(ch + 1) * FC])
                    nc.tensor.matmul(q_ps, ones, sq,
                                     start=(ko == 0), stop=(ko == KO - 1))
                mu = ln1p.tile([1, FC], f32, tag="mu")
                nc.vector.tensor_scalar(out=mu, in0=s_ps, scalar1=1.0 / D,
                                        scalar2=None, op0=ALU.mult)
                var = ln1p.tile([1, FC], f32, tag="var")
                nc.vector.tensor_scalar(out=var, in0=q_ps, scalar1=1.0 / D,
                                        scalar2=None, op0=ALU.mult)
                tmp = ln1p.tile([1, FC], f32, tag="tmp")
                nc.vector.tensor_mul(out=tmp, in0=mu, in1=mu)
                nc.vector.tensor_tensor(out=var, in0=var, in1=tmp,
                                        op=ALU.subtract)
                nc.vector.tensor_scalar(out=var, in0=var, scalar1=EPS,
                                        scalar2=None, op0=ALU.add)
                nc.scalar.activation(out=var, in_=var, func=AF.Sqrt)
                nc.vector.reciprocal(out=tmp, in_=var)       # tmp = rstd
                nc.vector.tensor_mul(out=mu, in0=mu, in1=tmp)
                nc.vector.tensor_scalar(out=mu, in0=mu, scalar1=-1.0,
                                        scalar2=None, op0=ALU.mult)  # mu = -mu*rstd
                rb16 = ln1p.tile([1, FC], bf16, tag="rb16")
                nc.vector.tensor_copy(out=rb16, in_=tmp)
                nb16 = ln1p.tile([1, FC], bf16, tag="nb16")
                nc.vector.tensor_copy(out=nb16, in_=mu)
                # broadcast across the 128 partitions via DRAM bounce
                bcs = io["bc_scratch"]
                nc.sync.dma_start(out=bcs.ap()[0:1, ch * FC:(ch + 1) * FC],
                                  in_=rb16)
                nc.sync.dma_start(out=bcs.ap()[1:2, ch * FC:(ch + 1) * FC],
                                  in_=nb16)
                nc.sync.dma_start(
                    out=r_bc[:, ch * FC:(ch + 1) * FC],
                    in_=bass.AP(tensor=bcs, offset=ch * FC,
                                ap=[[0, P], [1, FC]]))
                nc.sync.dma_start(
                    out=nmr_bc[:, ch * FC:(ch + 1) * FC],
                    in_=bass.AP(tensor=bcs, offset=T + ch * FC,
                                ap=[[0, P], [1, FC]]))
            # xnT = xbT * r + (-mu*r), chunked for pipelining with phase B
            for ch in range(T // FC):
                s = slice(ch * FC, (ch + 1) * FC)
                for ko in range(KO):
                    eng = nc.gpsimd if ko < 2 else nc.vector
                    eng.tensor_mul(out=xnT[:, ko, s], in0=xbT[:, ko, s],
                                   in1=r_bc[:, s])
                    eng.tensor_add(out=xnT[:, ko, s], in0=xnT[:, ko, s],
                                   in1=nmr_bc[:, s])
        pA2_cm.__exit__(None, None, None)

        if _ONLY_A:
            with tc.tile_pool(name="dumA", bufs=2) as dp:
                for qt in range(NQT):
                    t0 = dp.tile([P, KO, P], f32, tag="t0")
                    nc.vector.tensor_copy(out=t0, in_=xnT[:, :, qt * P:(qt + 1) * P])
                    nc.sync.dma_start(
                        out=io["out"].ap()[qt * P:(qt + 1) * P, :],
                        in_=t0.rearrange("p ko t -> p (ko t)"))
            pABC_cm.__exit__(None, None, None)
            return

        # ---------- Phase B: Q^T, V rows, then K^T (fp8 DoubleRow) ----------
        pB2_cm = tc.tile_pool(name="pB2", bufs=1)
        pB2 = pB2_cm.__enter__()
        wv_sb = pB2.tile([P, KO // 2, 2, D], f8)
        nc.sync.dma_start(out=wv_sb, in_=io["w_v"].ap())
        bk_sb = pB2.tile([64, 16], f32)
        nc.sync.dma_start(out=bk_sb, in_=io["b_k"].ap())
        bq_sb = pB2.tile([64, 16], f32)
        nc.sync.dma_start(out=bq_sb, in_=io["b_q"].ap())
        wk_sb = pB2.tile([P, 16, KO // 2, 2, 64], f8)
        nc.sync.dma_start(out=wk_sb, in_=io["w_k"].ap())
        wq_sb = pB2.tile([P, 16, KO // 2, 2, 64], f8)
        nc.sync.dma_start(out=wq_sb, in_=io["w_q"].ap())
        xn8 = pB2.tile([P, KO, T], f8)

        with tc.tile_pool(name="wkv", bufs=3) as wp, \
             tc.tile_pool(name="kst", bufs=3) as kstp:
            # quantize xn to fp8 for the K DoubleRow matmuls (Act is idle here)
            for ko in range(KO):
                for ch in range(T // FC):
                    nc.scalar.copy(out=xn8[:, ko, ch * FC:(ch + 1) * FC],
                                   in_=xnT[:, ko, ch * FC:(ch + 1) * FC])
            for t64 in range(NT * 2):  # V rows, fp8 DoubleRow (64-tok tiles)
                tt, hi = t64 // 2, t64 % 2
                for vc in range(2):
                    acc = psB.tile([64, FC], f32, tag="t2")
                    for kp in range(KO // 2):
                        nc.tensor.matmul(
                            acc,
                            xn8[:, 2 * kp:2 * kp + 2,
                                t64 * 64:(t64 + 1) * 64],
                            wv_sb[:, kp, :, vc * FC:(vc + 1) * FC],
                            start=(kp == 0), stop=(kp == KO // 2 - 1),
                            perf_mode=DR)
                    hs = vc * 8
                    if hi == 0:
                        nc.scalar.activation(
                            out=v_aug[0:64, tt, hs:hs + 8, 0:HD],
                            in_=acc.rearrange("p (h d) -> p h d", h=8),
                            func=AF.Identity, scale=1.0 / 256.0)
                    else:
                        vst = kstp.tile([64, FC], bf16, tag="vst")
                        nc.vector.tensor_scalar(
                            out=vst, in0=acc, scalar1=1.0 / 256.0,
                            scalar2=None, op0=ALU.mult)
                        nc.sync.dma_start(
                            out=v_aug[64:128, tt, hs:hs + 8, 0:HD],
                            in_=vst.rearrange("p (h d) -> p h d", h=8))
            # K^T via fp8 DoubleRow: 16 ct64 tiles, odd tiles DMA-shifted
            # into partitions 64-127 of kT
            for ct in range(16):
                ct128, hi = ct // 2, ct % 2
                # interleaved Q^T (fp8 DR) keeps the PE stream dense
                qacc = psB.tile([64, TC], f32, tag="t2")
                for kp in range(KO // 2):
                    nc.tensor.matmul(qacc, wq_sb[:, ct, kp, :, :],
                                     xn8[:, 2 * kp:2 * kp + 2, 0:TC],
                                     start=(kp == 0), stop=(kp == KO // 2 - 1),
                                     perf_mode=DR)
                if hi == 0:
                    nc.scalar.activation(out=qT[0:64, ct128, :], in_=qacc,
                                         func=AF.Identity,
                                         bias=bq_sb[:, ct:ct + 1],
                                         scale=1.0 / 256.0)
                else:
                    qst = kstp.tile([64, TC], bf16, tag="qst")
                    nc.vector.tensor_scalar(
                        out=qst, in0=qacc, scalar1=1.0 / 256.0,
                        scalar2=bq_sb[:, ct:ct + 1],
                        op0=ALU.mult, op1=ALU.add)
                    nc.sync.dma_start(out=qT[64:128, ct128, :], in_=qst)
                for np_ in range(T // FC):
                    acc = psA.tile([64, FC], f32, tag="t1")
                    for kp in range(KO // 2):
                        nc.tensor.matmul(
                            acc, wk_sb[:, ct, kp, :, :],
                            xn8[:, 2 * kp:2 * kp + 2,
                                np_ * FC:(np_ + 1) * FC],
                            start=(kp == 0), stop=(kp == KO // 2 - 1),
                            perf_mode=DR)
                    if hi == 0:
                        nc.scalar.activation(
                            out=kT[0:64, ct128, np_ * FC:(np_ + 1) * FC],
                            in_=acc, func=AF.Identity,
                            bias=bk_sb[:, ct:ct + 1], scale=1.0 / 256.0)
                    else:
                        kst = kstp.tile([64, FC], bf16, tag="kst")
                        nc.vector.tensor_scalar(
                            out=kst, in0=acc, scalar1=1.0 / 256.0,
                            scalar2=bk_sb[:, ct:ct + 1],
                            op0=ALU.mult, op1=ALU.add)
                        nc.sync.dma_start(
                            out=kT[64:128, ct128, np_ * FC:(np_ + 1) * FC],
                            in_=kst)
        pB2_cm.__exit__(None, None, None)
        pXN_cm.__exit__(None, None, None)
        for qt in range(NQT):
            nc.sync.dma_start(out=xo_all[:, qt, :],
                              in_=io["x_own"].ap()[qt * P:(qt + 1) * P, :])

        # ---------- Phase C: attention ----------
        pC2_cm = tc.tile_pool(name="pC2", bufs=1)
        pC2 = pC2_cm.__enter__()
        mask_sb = pC2.tile([P, NKT, TC], bf16)
        nc.sync.dma_start(out=mask_sb, in_=io["mask"].ap())
        with tc.tile_pool(name="attp", bufs=2) as attp:
            for h in range(NH):
                hp = (h * HD) % P
                hko = (h * HD) // P
                attT = attp.tile([P, NKT, TC], bf16)
                for kp in range(NKT // 2):
                    sc = psA.tile([P, 2, TC], f32, tag="t1")
                    for half in range(2):
                        kt = kp * 2 + half
                        nc.tensor.matmul(sc[:, half, :],
                                         kT[hp:hp + HD, hko, kt * P:(kt + 1) * P],
                                         qT[hp:hp + HD, hko, :],
                                         start=True, stop=True)
                    nc.scalar.activation(out=attT[:, kp * 2:(kp + 1) * 2, :],
                                         in_=sc, func=AF.Exp)
                nc.vector.tensor_mul(out=attT[:, :, :], in0=attT[:, :, :],
                                     in1=mask_sb[:, :, :])
                for qt in range(NQT):
                    av = psB.tile([P, HD + 1], f32, tag="t2")
                    for kt in range(NKT):
                        nc.tensor.matmul(av,
                                         attT[:, kt, qt * P:(qt + 1) * P],
                                         v_aug[:, kt, h, :],
                                         start=(kt == 0), stop=(kt == NKT - 1))
                    recip = attp.tile([P, 1], f32, tag="recip")
                    nc.vector.reciprocal(out=recip, in_=av[:, HD:HD + 1])
                    nc.vector.tensor_scalar(
                        out=attn_out[:, qt, h * HD:(h + 1) * HD],
                        in0=av[:, 0:HD], scalar1=recip, scalar2=None, op0=ALU.mult)

        pC2_cm.__exit__(None, None, None)
        pABC_cm.__exit__(None, None, None)

        # ---------- Phase D: proj + residual -> x2; ln2 -> xn2T ----------
        pDE_cm = tc.tile_pool(name="pDE", bufs=1)
        pDE = pDE_cm.__enter__()
        pE_cm = tc.tile_pool(name="pE", bufs=1)
        pE = pE_cm.__enter__()
        pW_cm = tc.tile_pool(name="pW", bufs=1)
        pW = pW_cm.__enter__()
        x2 = pDE.tile([P, NQT, D], f32)
        xn2T = pDE.tile([P, KO, TC], bf16)
        wproj_sb = pW.tile([P, KO, D], bf16)
        nc.sync.dma_start(out=wproj_sb, in_=io["w_proj"].ap())
        w1 = pE.tile([P, NFT, KO, P], bf16)
        nc.sync.dma_start(out=w1, in_=io["fc1_w"].ap())
        w2a = pE.tile([P, NFP, 2, FC], f8)
        nc.sync.dma_start(out=w2a, in_=io["fc2_w"].ap()[:, 0])
        w2b = pE.tile([P, NFP, 2, FC], f8)
        nc.sync.dma_start(out=w2b, in_=io["fc2_w"].ap()[:, 1])
        b1_sb = pE.tile([P, NFT], f32)
        nc.sync.dma_start(out=b1_sb, in_=io["fc1_b"].ap())
        b2rep = pE.tile([P, D], f32)
        nc.sync.dma_start(out=b2rep, in_=io["fc2_b_rep"].ap())
        eps_t = pDE.tile([P, 1], f32)
        nc.vector.memset(eps_t, EPS)

        with tc.tile_pool(name="projp", bufs=3) as pp:
            attn_outT = pW.tile([P, KO, TC], bf16)
            for qt in range(NQT):
                for ko in range(KO):
                    tp = psA.tile([P, P], bf16, tag="t1")
                    nc.tensor.transpose(
                        tp, attn_out[:, qt, ko * P:(ko + 1) * P], identb)
                    nc.vector.tensor_copy(
                        out=attn_outT[:, ko, qt * P:(qt + 1) * P], in_=tp)
            for qt in range(NQT):
                xo = xo_all[:, qt, :]
                for oc in range(D // FC):
                    acc = psB.tile([P, FC], f32, tag="t2")
                    for ko in range(KO):
                        nc.tensor.matmul(acc, attn_outT[:, ko, qt * P:(qt + 1) * P],
                                         wproj_sb[:, ko, oc * FC:(oc + 1) * FC],
                                         start=(ko == 0), stop=(ko == KO - 1))
                    nc.vector.tensor_add(out=x2[:, qt, oc * FC:(oc + 1) * FC],
                                         in0=acc, in1=xo[:, oc * FC:(oc + 1) * FC])
                stats = pp.tile([P, 2, 6], f32, tag="st2")
                nc.vector.bn_stats(out=stats[:, 0, :], in_=x2[:, qt, 0:512])
                nc.vector.bn_stats(out=stats[:, 1, :], in_=x2[:, qt, 512:1024])
                mv = pp.tile([P, 2], f32, tag="mv2")
                nc.vector.bn_aggr(out=mv, in_=stats)
                std = pp.tile([P, 1], f32, tag="sd2")
                nc.scalar.activation(out=std, in_=mv[:, 1:2], func=AF.Sqrt,
                                     bias=eps_t, scale=1.0)
                rstd = pp.tile([P, 1], f32, tag="rs2")
                nc.vector.reciprocal(out=rstd, in_=std)
                xn2 = pp.tile([P, D], bf16, tag="xn2")
                nc.vector.tensor_scalar(out=xn2, in0=x2[:, qt, :], scalar1=mv[:, 0:1],
                                        scalar2=rstd, op0=ALU.subtract, op1=ALU.mult)
                for ko in range(KO):
                    tp = psA.tile([P, P], bf16, tag="t1")
                    nc.tensor.transpose(tp, xn2[:, ko * P:(ko + 1) * P], identb)
                    nc.vector.tensor_copy(
                        out=xn2T[:, ko, qt * P:(qt + 1) * P], in_=tp)

        pW_cm.__exit__(None, None, None)

        psB_cm.__exit__(None, None, None)
        psA_cm.__exit__(None, None, None)
        # ---------- Phase E: fc1 -> gelu -> hT; fc2 (fp8 DoubleRow) ----------
        hT = pE.tile([P, NFT, TC], f8)
        x2b = pE.tile([P, NQT, D], bf16)
        for qt in range(NQT):
            for oc in range(2):
                nc.vector.scalar_tensor_tensor(
                    out=x2b[:, qt, oc * FC:(oc + 1) * FC],
                    in0=x2[:, qt, oc * FC:(oc + 1) * FC],
                    scalar=256.0, op0=ALU.mult,
                    in1=b2rep[:, oc * FC:(oc + 1) * FC], op1=ALU.add)
        with tc.tile_pool(name="f1ps", bufs=4, space="PSUM") as f1ps:
            for ft in range(NFT):
                acc = f1ps.tile([P, TC], f32, tag="facc1")
                for ko in range(KO):
                    nc.tensor.matmul(acc, w1[:, ft, ko, :], xn2T[:, ko, :],
                                     start=(ko == 0), stop=(ko == KO - 1))
                nc.scalar.activation(out=hT[:, ft, :], in_=acc, func=AF.Gelu,
                                     bias=b1_sb[:, ft:ft + 1], scale=1.0)

        with tc.tile_pool(name="fc2p", bufs=4, space="PSUM") as f2ps, \
             tc.tile_pool(name="fc2o", bufs=4) as f2p:
            for oc in range(2):
                w2 = w2a if oc == 0 else w2b
                for q64 in range(TC // 64):
                    acc = f2ps.tile([64, FC], f32, tag="facc")
                    for i in range(NFP):
                        nc.tensor.matmul(
                            acc,
                            hT[:, 2 * i:2 * i + 2, q64 * 64:(q64 + 1) * 64],
                            w2[:, i, :, :],
                            start=(i == 0), stop=False,
                            perf_mode=DR)
                    qt, h2 = q64 // 2, q64 % 2
                    nc.tensor.matmul(
                        acc,
                        identb[h2 * 64:h2 * 64 + 64, h2 * 64:h2 * 64 + 64],
                        x2b[h2 * 64:h2 * 64 + 64, qt, oc * FC:(oc + 1) * FC],
                        start=False, stop=True)
                    osb = f2p.tile([64, FC], f32, tag="osb")
                    nc.vector.tensor_scalar(out=osb, in0=acc, scalar1=1.0 / 256.0,
                                            scalar2=None, op0=ALU.mult)
                    nc.sync.dma_start(
                        out=io["out"].ap()[q64 * 64:(q64 + 1) * 64,
                                           oc * FC:(oc + 1) * FC],
                        in_=osb)
        pE_cm.__exit__(None, None, None)
        pDE_cm.__exit__(None, None, None)


def _stage_inputs(x, w_qkv, w_proj, ln1_w, ln1_b, ln2_w, ln2_b,
                  fc1_w, fc1_b, fc2_w, fc2_b):
    """Build the 8 per-core input maps (host-side sharding/folding/tiling)."""
    f = np.float32
    bf = ml_dtypes.bfloat16
    x = np.asarray(x, f)
    w_qkv = np.asarray(w_qkv, f)
    ln1_w, ln1_b = np.asarray(ln1_w, f), np.asarray(ln1_b, f)
    ln2_w, ln2_b = np.asarray(ln2_w, f), np.asarray(ln2_b, f)
    fc1_wf, fc1_bf = np.asarray(fc1_w, f), np.asarray(fc1_b, f)
    fc2_wf, fc2_bf = np.asarray(fc2_w, f), np.asarray(fc2_b, f)
    w_projf = np.asarray(w_proj, f)

    wq_f = ln1_w[:, None] * w_qkv
    bq_f = ln1_b @ w_qkv
    scale = 1.0 / np.sqrt(HD)

    def tile_cols(w, dtype):
        # [D, M] -> [M/P, P, KO, P]: out[ct, p, ko, m] = w[ko*P+p, ct*P+m]
        Din, M = w.shape
        return np.ascontiguousarray(
            w.reshape(KO, P, M // P, P).transpose(2, 1, 0, 3)).astype(dtype)

    f8q = ml_dtypes.float8_e4m3fn
    w_q_h = np.ascontiguousarray(
        (wq_f[:, 0:D] * scale * 256.0).reshape(KO // 2, 2, P, 16, 64)
        .transpose(2, 3, 0, 1, 4)).astype(f8q)
    b_q_h = np.ascontiguousarray(
        (bq_f[0:D] * scale).reshape(16, 64).T).astype(f)
    f8np_ = ml_dtypes.float8_e4m3fn
    # K weights as fp8 DoubleRow tiles [p, ct64, kp, j, m], scaled x256
    w_k_h = np.ascontiguousarray(
        (wq_f[:, D:2 * D] * 256.0).reshape(KO // 2, 2, P, 16, 64)
        .transpose(2, 3, 0, 1, 4)).astype(f8np_)
    b_k_h = np.ascontiguousarray(bq_f[D:2 * D].reshape(16, 64).T).astype(f)
    # V weights fp8 DR [p, kp, j, c], scaled x256
    w_v_h = np.ascontiguousarray(
        (wq_f[:, 2 * D:3 * D] * 256.0).reshape(KO // 2, 2, P, D)
        .transpose(2, 0, 1, 3)).astype(ml_dtypes.float8_e4m3fn)
    b_v = bq_f[2 * D:3 * D]
    bvproj = b_v @ w_projf

    w_proj_h = np.ascontiguousarray(
        w_projf.reshape(KO, P, D).transpose(1, 0, 2)).astype(bf)
    f8np = ml_dtypes.float8_e4m3fn
    fc1_w_h = np.ascontiguousarray(
        tile_cols(ln2_w[:, None] * fc1_wf, bf).transpose(1, 0, 2, 3))
    fc1_b_h = np.ascontiguousarray(
        (ln2_b @ fc1_wf + fc1_bf).reshape(NFT, P).T).astype(f)
    # fc2 DoubleRow tiles [i, oc, P, 2, FC]:
    #   fc2_h[i, oc, p, j, m] = fc2[(2*i+j)*P + p, oc*FC + m]
    # [p, oc, i, j, c] = fc2[(2i+j)*P + p, oc*FC + c]
    fc2_h = np.ascontiguousarray(
        fc2_wf.reshape(NFP, 2, P, 2, FC).transpose(2, 3, 0, 1, 4) * 256.0).astype(f8np)
    fc2_b_rep_h = np.ascontiguousarray(
        np.broadcast_to(fc2_bf * 256.0, (P, D))).astype(f)
    eye = np.eye(P, dtype=f)
    eyeb = np.eye(P, dtype=f).astype(bf)
    ones_h = np.ones((P, 1), f).astype(bf)

    in_maps = []
    for c in range(N_CORES):
        b = c // CPB
        r0 = (c % CPB) * TC
        xb_c = np.roll(x[b], -r0, axis=0)
        xbT_c = np.ascontiguousarray(xb_c.T).astype(bf)
        x_own_c = (x[b, r0:r0 + TC] + bvproj).astype(f)
        kidx = (np.arange(T) + r0) % T
        qidx = r0 + np.arange(TC)
        m = (kidx[:, None] <= qidx[None, :])
        mask_c = np.ascontiguousarray(
            m.reshape(NKT, P, TC).transpose(1, 0, 2)).astype(bf)
        in_maps.append({
            "xbT": xbT_c,
            "x_own": x_own_c,
            "w_k": w_k_h,
            "w_v": w_v_h,
            "w_q": w_q_h,
            "b_k": b_k_h,
            "b_q": b_q_h,
            "w_proj": w_proj_h,
            "fc1_w": fc1_w_h,
            "fc1_b": fc1_b_h,
            "fc2_w": fc2_h,
            "fc2_b_rep": fc2_b_rep_h,
            "mask": mask_c,
            "ident": eye,
            "identb": eyeb,
            "ones": ones_h,
        })
    return in_maps


def kernel(**inputs) -> np.ndarray:
    if "nc" not in _CACHE:
        _CACHE["nc"] = build_nc()
    nc = _CACHE["nc"]
    in_maps = _stage_inputs(**inputs)
    res = run_bass_kernel_spmd(nc, in_maps, list(range(N_CORES)))
    out = np.empty((B, T, D), np.float32)
    for c in range(N_CORES):
        b = c // CPB
        r0 = (c % CPB) * TC
        out[b, r0:r0 + TC] = res.results[c]["out"]
    return out

